# revision 20
# baseline (speedup 1.0000x reference)
"""Fused additive-attention kernel for Trainium2 (8 NeuronCores, SPMD).

Computes  w = softmax_K( mask ? (Wl . tanh(vW_v^T + qW_q^T) + bl) : -1e9 )
without ever materializing the [B,N,S,K,H] joint_repr intermediate.

Sharding: data-parallel over batch B (16) across 8 cores -> 2 batches/core.
Weights replicated. Host does layout prep only (transposes / packing); all
FLOPs (matmuls, tanh, softmax) run on device.

Per-core dataflow (h on partitions for the hot loop), phased over h-chunk
pairs so compute starts before all weights arrive:
  qpT[hc]  [128(h), 512(b,ns)]   = WqT-slice.T @ qT      (PE, psum acc over qd)
  vpT[hc]  [128(h), 100(b,k)]    = WvT-slice.T @ vT      (PE)
  JT       [128, (hc2,k10,b,ns)] = qpT + vp[b,k] col     (DVE tensor_scalar 4x bf16)
  TH       = tanh(JT)  bf16      one [128, 10240] ACT op per k-group
  logits   psum [50(k), 512(b,ns)] += WlZ[hc,j(k)].T @ TH-slice (PE; WlZ is
           zero-padded so row k accumulates only its own contribution)
  masked softmax over k after PE-transposing logits to [ns, k].
"""

import os
import sys

import numpy as np

sys.path.insert(0, "/opt/trn_rl_repo")

import concourse.bass as bass
import concourse.mybir as mybir
from concourse import bacc, bass_utils
from concourse.tile import TileContext

# Problem shapes (hardcoded per contract -- kernel.py must be self-contained)
B, N, S, K = 16, 4, 64, 50
VD, QD, H = 1024, 768, 512
NCORES = 8
BPC = B // NCORES          # batches per core = 2
NS = BPC * N * S           # 512 rows (b, n, s) per core
KB = BPC * K               # 100 (b, k) columns per core
HC = H // 128              # 4 h-chunks
QC = QD // 128             # 6 qd-chunks
VC = VD // 128             # 8 vd-chunks
G = 10                     # k's per staging group
NG = K // G                # 5 groups per hc-phase

F32 = mybir.dt.float32
BF16 = mybir.dt.bfloat16

_CACHE = {}


def _build_nc():
    nc = bacc.Bacc("TRN2", target_bir_lowering=False)

    qT_h = nc.dram_tensor("qT", [QD, NS], F32, kind="ExternalInput")
    vT_h = nc.dram_tensor("vT", [VD, KB], F32, kind="ExternalInput")
    # weight slabs, pre-split by hc-pair: A = h cols 0:256, B = 256:512
    WqTA_h = nc.dram_tensor("WqTA", [QD, 256], F32, kind="ExternalInput")
    WqTB_h = nc.dram_tensor("WqTB", [QD, 256], F32, kind="ExternalInput")
    WvTA_h = nc.dram_tensor("WvTA", [VD, 256], F32, kind="ExternalInput")
    WvTB_h = nc.dram_tensor("WvTB", [VD, 256], F32, kind="ExternalInput")
    # packed [128, 12]: cols 0:4 Wl chunks, 4:8 bq chunks, 8:12 bv chunks
    wlb_h = nc.dram_tensor("wlb", [128, 12], F32, kind="ExternalInput")
    # zero-padded Wl variants: [128, hc*2500 + j*50 + c] = Wl[hc*128+p]*(c==j)
    wlz_h = nc.dram_tensor("wlz", [128, HC * K * K], BF16, kind="ExternalInput")
    # packed [128, 200]: cols 0:100 maskf (b,k) replicated, 100:200 (maskf-1)*1e9
    msk_h = nc.dram_tensor("msk", [128, 2 * KB], F32, kind="ExternalInput")
    id_h = nc.dram_tensor("ident", [128, 128], F32, kind="ExternalInput")
    out_h = nc.dram_tensor("out", [NS, K], F32, kind="ExternalOutput")

    with TileContext(nc) as tc:
        with (
            tc.tile_pool(name="persist", bufs=1) as pp,
            tc.tile_pool(name="ppsum", bufs=1, space="PSUM") as ppsum,
            tc.tile_pool(name="smpsum", bufs=2, space="PSUM") as sps,
        ):
            # ---- DMA loads, ordered so phase A can start early ----
            qts = pp.tile([128, QC, NS], F32, name="qts")
            nc.sync.dma_start(
                qts[:, :, :], qT_h[:, :].rearrange("(c p) j -> p c j", p=128)
            )
            wqtA = pp.tile([128, QC, 256], F32, name="wqtA")
            nc.sync.dma_start(
                wqtA[:, :, :], WqTA_h[:, :].rearrange("(c p) j -> p c j", p=128)
            )
            vts = pp.tile([128, VC, KB], F32, name="vts")
            nc.sync.dma_start(
                vts[:, :, :], vT_h[:, :].rearrange("(c p) j -> p c j", p=128)
            )
            wvtA = pp.tile([128, VC, 256], F32, name="wvtA")
            nc.sync.dma_start(
                wvtA[:, :, :], WvTA_h[:, :].rearrange("(c p) j -> p c j", p=128)
            )
            wlb = pp.tile_from(wlb_h[:, :], name="wlb")
            wlz = pp.tile_from(wlz_h[:, :], name="wlz")
            msk = pp.tile_from(msk_h[:, :], name="msk")
            ident = pp.tile_from(id_h[:, :], name="ident")
            wqtB = pp.tile([128, QC, 256], F32, name="wqtB")
            nc.sync.dma_start(
                wqtB[:, :, :], WqTB_h[:, :].rearrange("(c p) j -> p c j", p=128)
            )
            wvtB = pp.tile([128, VC, 256], F32, name="wvtB")
            nc.sync.dma_start(
                wvtB[:, :, :], WvTB_h[:, :].rearrange("(c p) j -> p c j", p=128)
            )

            # qpT (all h-chunks): [128, (hc, b, ns)] bf16; vpT: [128,(hc,b,k)] f32
            QPs = pp.tile([128, HC * NS], BF16, name="QPs")
            VPs = pp.tile([128, HC * KB], F32, name="VPs")

            # logits psum [50(k), 512(b,ns)] -- single accumulation group
            ps_log = ppsum.tile([K, NS], F32, name="ps_log")

            def proj_phase(ph, wqt, wvt):
                """Compute QPs/VPs h-chunks [2*ph, 2*ph+2) from slab wqt/wvt."""
                with tc.tile_pool(name=f"p1ps{ph}", bufs=1, space="PSUM") as p1ps:
                    pq = [
                        p1ps.tile([128, NS], F32, name=f"pq{ph}{i}")
                        for i in range(2)
                    ]
                    for qc in range(QC):
                        for i in range(2):
                            nc.tensor.matmul(
                                pq[i][:, :],
                                wqt[:, qc, i * 128 : (i + 1) * 128],
                                qts[:, qc, :],
                                start=(qc == 0),
                                stop=(qc == QC - 1),
                            )
                    pv = [
                        p1ps.tile([128, KB], F32, name=f"pv{ph}{i}")
                        for i in range(2)
                    ]
                    for vc in range(VC):
                        for i in range(2):
                            nc.tensor.matmul(
                                pv[i][:, :],
                                wvt[:, vc, i * 128 : (i + 1) * 128],
                                vts[:, vc, :],
                                start=(vc == 0),
                                stop=(vc == VC - 1),
                            )
                    for i in range(2):
                        hc = 2 * ph + i
                        nc.vector.tensor_scalar_add(
                            QPs[:, hc * NS : (hc + 1) * NS],
                            pq[i][:, :],
                            wlb[:, HC + hc : HC + hc + 1],
                        )
                        nc.vector.tensor_scalar_add(
                            VPs[:, hc * KB : (hc + 1) * KB],
                            pv[i][:, :],
                            wlb[:, 2 * HC + hc : 2 * HC + hc + 1],
                        )

            def main_phase(ph, mp, mid_cb=None):
                """Joint tanh + logit matmuls for h-chunks {2ph, 2ph+1}."""
                for g in range(NG):
                    if g == 1 and mid_cb is not None:
                        mid_cb()
                    JT = mp.tile([128, 2 * G * NS], BF16, tag="JT", name="JT")
                    TH = mp.tile([128, 2 * G * NS], BF16, tag="TH", name="TH")
                    for i in range(2):
                        hc = 2 * ph + i
                        for kk in range(G):
                            k = g * G + kk
                            for b in range(BPC):
                                off = i * G * NS + kk * NS + b * (NS // BPC)
                                nc.vector.tensor_scalar_add(
                                    JT[:, off : off + NS // BPC],
                                    QPs[
                                        :,
                                        hc * NS
                                        + b * (NS // BPC) : hc * NS
                                        + (b + 1) * (NS // BPC),
                                    ],
                                    VPs[
                                        :,
                                        hc * KB + b * K + k : hc * KB
                                        + b * K
                                        + k
                                        + 1,
                                    ],
                                )
                        # tanh per hc-half keeps PE fed every ~4.4us (HAM warm)
                        nc.scalar.activation(
                            TH[:, i * G * NS : (i + 1) * G * NS],
                            JT[:, i * G * NS : (i + 1) * G * NS],
                            mybir.ActivationFunctionType.Tanh,
                        )
                        for kk in range(G):
                            k = g * G + kk
                            nc.tensor.matmul(
                                ps_log[:, :],
                                wlz[:, hc * K * K + k * K : hc * K * K + (k + 1) * K],
                                TH[:, i * G * NS + kk * NS : i * G * NS + (kk + 1) * NS],
                                start=(ph == 0 and g == 0 and i == 0 and kk == 0),
                                stop=(ph == 1 and g == NG - 1 and i == 1 and kk == G - 1),
                                skip_group_check=True,
                            )

            def proj_b():
                with tc.high_priority():
                    proj_phase(1, wqtB, wvtB)

            proj_phase(0, wqtA, wvtA)
            with tc.tile_pool(name="main", bufs=2) as mp:
                main_phase(0, mp, mid_cb=proj_b)
                main_phase(1, mp)

            # ---- masked softmax over k ----
            LG = pp.tile([K, NS], F32, name="LG")
            W_all = pp.tile([128, NS // 128, K], F32, name="W_all")
            nc.vector.tensor_copy(LG[:, :], ps_log[:, :])
            for nsc in range(NS // 128):
                b = nsc // ((NS // BPC) // 128)
                ps_t = sps.tile([128, K], F32, tag="ps_t", name="ps_t")
                nc.tensor.transpose(
                    ps_t[:, :],
                    LG[:, nsc * 128 : (nsc + 1) * 128],
                    ident[0:K, 0:K],
                )
                LT = pp.tile([128, K], F32, name=f"LT{nsc}")
                nc.vector.tensor_copy(LT[:, :], ps_t[:, :])
                # masked = logits*maskf + (maskf-1)*1e9
                nc.vector.tensor_mul(
                    LT[:, :], LT[:, :], msk[:, b * K : (b + 1) * K]
                )
                nc.vector.tensor_add(
                    LT[:, :], LT[:, :], msk[:, KB + b * K : KB + (b + 1) * K]
                )
                mx = pp.tile([128, 1], F32, name=f"mx{nsc}")
                nc.vector.tensor_reduce(
                    mx[:, :], LT[:, :], axis=mybir.AxisListType.X,
                    op=mybir.AluOpType.max,
                )
                mxn = pp.tile([128, 1], F32, name=f"mxn{nsc}")
                nc.vector.tensor_scalar_mul(mxn[:, :], mx[:, :], -1.0)
                EX = pp.tile([128, K], F32, name=f"EX{nsc}")
                sm = pp.tile([128, 1], F32, name=f"sm{nsc}")
                nc.scalar.activation(
                    EX[:, :], LT[:, :], mybir.ActivationFunctionType.Exp,
                    bias=mxn[:, 0:1], accum_out=sm[:, 0:1],
                )
                rs = pp.tile([128, 1], F32, name=f"rs{nsc}")
                nc.vector.reciprocal(rs[:, :], sm[:, :])
                nc.vector.tensor_scalar_mul(
                    W_all[:, nsc, :], EX[:, :], rs[:, 0:1]
                )
            nc.sync.dma_start(
                out_h[:, :].rearrange("(c p) j -> p c j", p=128), W_all[:, :, :]
            )

    nc.finalize()
    return nc


def _prep_in_maps(v, q, box_mask, Wv, bv, Wq, bq, Wl):
    """Host-side layout prep: shard over B, transpose to device layouts."""
    import ml_dtypes

    v = np.asarray(v, np.float32).reshape(B, K, VD)
    q = np.asarray(q, np.float32).reshape(B, N * S, QD)
    mask = np.asarray(box_mask).astype(np.float32).reshape(B, K)

    WqT = np.asarray(Wq, np.float32).T                                # [QD, H]
    WvT = np.asarray(Wv, np.float32).T                                # [VD, H]
    WqTA = np.ascontiguousarray(WqT[:, :256])
    WqTB = np.ascontiguousarray(WqT[:, 256:])
    WvTA = np.ascontiguousarray(WvT[:, :256])
    WvTB = np.ascontiguousarray(WvT[:, 256:])
    wlb = np.zeros((128, 12), np.float32)
    wl_chunks = np.asarray(Wl, np.float32).reshape(4, 128).T          # [128, hc]
    wlb[:, 0:4] = wl_chunks
    wlb[:, 4:8] = np.asarray(bq, np.float32).reshape(4, 128).T
    wlb[:, 8:12] = np.asarray(bv, np.float32).reshape(4, 128).T
    # zero-padded Wl variants: wlz[p, hc*2500 + j*50 + c] = Wl_chunk[p,hc]*(c==j)
    wlz = np.zeros((128, HC, K, K), np.float32)
    for j in range(K):
        wlz[:, :, j, j] = wl_chunks
    wlz = wlz.reshape(128, HC * K * K).astype(ml_dtypes.bfloat16)
    ident = np.eye(128, dtype=np.float32)

    in_maps = []
    for c in range(NCORES):
        b0 = c * BPC
        qc = q[b0 : b0 + BPC].reshape(NS, QD)
        vc = v[b0 : b0 + BPC].reshape(KB, VD)
        qT = np.ascontiguousarray(qc.T)                               # [QD, NS]
        vT = np.ascontiguousarray(vc.T)                               # [VD, KB]
        mf = mask[b0 : b0 + BPC].reshape(1, KB)
        msk = np.zeros((128, 2 * KB), np.float32)
        msk[:, :KB] = mf
        msk[:, KB:] = (mf - 1.0) * 1e9
        in_maps.append(
            {
                "qT": qT,
                "vT": vT,
                "WqTA": WqTA,
                "WqTB": WqTB,
                "WvTA": WvTA,
                "WvTB": WvTB,
                "wlb": wlb,
                "wlz": wlz,
                "msk": msk,
                "ident": ident,
            }
        )
    return in_maps


def kernel(v, q, box_mask, tags_attention, Wv, bv, Wq, bq, Wl, bl):
    # bl shifts all unmasked logits uniformly -> cancels in softmax.
    # tags_attention is unused by the reference module.
    if "nc" not in _CACHE:
        _CACHE["nc"] = _build_nc()
    nc = _CACHE["nc"]
    in_maps = _prep_in_maps(v, q, box_mask, Wv, bv, Wq, bq, Wl)
    res = bass_utils.run_bass_kernel_spmd(
        nc,
        in_maps,
        core_ids=list(range(NCORES)),
        trace=bool(os.environ.get("KERNEL_TRACE")),
        tmpdir=os.environ.get("KERNEL_TMPDIR"),
    )
    _CACHE["last_result"] = res
    outs = [r["out"].reshape(BPC, N, S, K) for r in res.results]
    return np.concatenate(outs, axis=0)
